# revision 56
# baseline (speedup 1.0000x reference)
"""TRN2 Bass/Tile kernel: BERT self-attention (B=2, S=2048, H=1024, 16 heads, d=64).

Sharding (host side, all 8 cores run one SPMD NEFF):
  core c: batch b = c // 4, head group g = c % 4 (heads 4g..4g+3 = weight cols
  256g..256g+256). Each core receives X^T [H, S] for its batch (host transpose,
  cast to bf16), plus its weight columns pre-rearranged per head-pair into the
  on-chip [128, chunk, 128] layout (contiguous 2KB DMA rows), and returns its
  [S, 256] fp32 slice of the output in natural orientation.

Device algorithm (per core), bf16 matmul inputs / fp32 PSUM accumulation:
  1. Projections: Q^T/K^T in [d-pair(128), pair, s] layout, V in natural
     [s, kt, head, d+1] layout with a constant-1 column (ones-augmented V).
     All matmuls stream >=128 output rows in bf16 (1 PE cycle/row). PSUM is
     evacuated to bf16 SBUF by the DVE engine so the ACT engine stays
     dedicated to exp (the global bottleneck: S*S*4heads/128 rows = ~110us
     of exp per core; everything else hides behind it).
  2. Scores: scoresT[k, q] = K Q^T per (q-block 512, k-tile 128), bf16,
     512 free rows/matmul, landing in a ring of [128, 3, 512] PSUM triples
     (2 bufs = 6 banks). exp fires on ACT over whole triples ([128, 1536]
     per instruction, possibly spanning q-block boundaries -- exp is
     elementwise) to amortize the fixed PSUM/SBUF access overhead, writing
     persistent bf16 E tiles.
  3. ctx[q, d+1] += E^T-stationary x V_aug accumulated over k in PSUM
     (col d = softmax denominator via the ones column). Each q-tile's
     accumulation group runs to completion before the next group starts in
     the same bank: `start` marks the whole 2KB PSUM bank pending-zero, so
     interleaved long-lived groups in one bank clobber each other.
  4. Normalize on DVE: reciprocal of the denominator column + broadcast
     multiply, then DMA the [q, 4x64] block to the fp32 output.

  Scheduling (the PE queue is in-order, so emission order is the schedule):
  - Startup processes (block, k-tile) pairs of the first q-blocks of BOTH
    heads of pair 0 diagonally in X-arrival order, with K/Q projection
    slices at 256-wide granularity woven in right before the k-tiles that
    need them; h1's q-block reuses the pair-0 projections so the exp stream
    starts at ~7us and stays fed while X streams in.
  - Steady state weaves the pair-1 projections into later blocks, and
    trailing ctx blocks are pumped one q-tile group (16 matmuls) at a time
    between kt-pairs so the PE never inserts a scores gap longer than ACT's
    one-triple backlog.
  - The final ctx block splits its k-range 12/4 across two PSUM banks so
    only the last 4 k-tiles' matmuls (plus a DVE combine) trail the last
    exp.
  - PE warm-up dummy matmuls at t=0 keep the p-state ramp from resetting
    during the DMA-gated start.

  Softmax skips the row-max subtraction (scores ~ N(0,1) after the 1/8
  scale; exp cannot overflow) and defers normalization to the ctx output.
  bf16 end-to-end relative error is ~5e-3 (well under the 2e-2 gate).

  The spec pins biases and attention_mask to zeros; nonzero values get a
  generic (slightly slower) variant selected at build time, zeros skip the
  work entirely.

  _split_multi_waits: this walrus build packs at most one sync-wait per
  instruction, so Tile's multi-wait instructions get their extra waits
  hoisted onto single-wait InstEventSemaphore carriers.
"""

import functools
import numpy as np

B_FULL = 2
S_FULL = 2048
H_FULL = 1024
NHEADS = 16
DHEAD = 64
NCORES = 8
CORES_PER_BATCH = 4
HEADS_PER_CORE = NHEADS // CORES_PER_BATCH  # 4

# Stash of the last BassKernelResults (test harness reads exec_time_ns off it).
LAST_RESULT = None


@functools.lru_cache(maxsize=None)
def _build(S, H, hpc, with_mask, with_bias):
    import concourse.bass as bass
    import concourse.tile as tile
    import concourse.mybir as mybir

    f32 = mybir.dt.float32
    bf16 = mybir.dt.bfloat16
    AF = mybir.ActivationFunctionType
    D = DHEAD
    HD = hpc * D            # output columns per core (256)
    NP = hpc // 2           # head pairs per core (2)
    HC = H // 128           # contraction chunks (8)
    SB = 512                # s-block for projections / q-block for attention
    NSB = S // SB           # 4
    KT = S // 128           # k-tiles (16)
    KP = KT // 2            # kt-pairs per attention block (8)
    QT = SB // 128          # q-tiles per q-block (4)
    assert S % SB == 0 and H % 128 == 0 and hpc % 2 == 0

    nc = bass.Bass()
    xt = nc.dram_tensor("xt", [H, S], bf16, kind="ExternalInput")
    # weights arrive host-rearranged to the on-chip layout, one tensor per
    # (matrix, head-pair): [partition(h%128), chunk(h//128), 128 out-cols]
    # so each pair's DMA moves contiguous 2KB rows (no small-row penalty)
    wqp = [nc.dram_tensor(f"wq{p}", [128, HC, 128], bf16,
                          kind="ExternalInput") for p in range(NP)]
    wkp = [nc.dram_tensor(f"wk{p}", [128, HC, 128], bf16,
                          kind="ExternalInput") for p in range(NP)]
    wvp = [nc.dram_tensor(f"wv{p}", [128, HC, 128], bf16,
                          kind="ExternalInput") for p in range(NP)]
    if with_bias:
        bq = nc.dram_tensor("bq", [HD], f32, kind="ExternalInput")
        bk = nc.dram_tensor("bk", [HD], f32, kind="ExternalInput")
        bv = nc.dram_tensor("bv", [HD], f32, kind="ExternalInput")
    msk = nc.dram_tensor("mask", [S], f32, kind="ExternalInput") if with_mask else None
    out = nc.dram_tensor("out", [S, HD], f32, kind="ExternalOutput")

    with tile.TileContext(nc) as tc:
        with tc.tile_pool(name="pers", bufs=1) as pers, \
             tc.tile_pool(name="pp", bufs=1, space="PSUM") as pp, \
             tc.tile_pool(name="psr", bufs=1, space="PSUM") as psr, \
             tc.tile_pool(name="cxp", bufs=1, space="PSUM") as cxp, \
             tc.tile_pool(name="ep", bufs=4) as ep, \
             tc.tile_pool(name="nrm", bufs=3) as nrm:
            # ---- persistent SBUF ----
            xts = pers.tile([128, HC, S], bf16, tag="xts", name="xts")
            wq_sbp = [pers.tile([128, HC, 128], bf16, tag=f"wq{p}",
                                name=f"wq_sb{p}") for p in range(NP)]
            wk_sbp = [pers.tile([128, HC, 128], bf16, tag=f"wk{p}",
                                name=f"wk_sb{p}") for p in range(NP)]
            wv_sbp = [pers.tile([128, HC, 128], bf16, tag=f"wv{p}",
                                name=f"wv_sb{p}") for p in range(NP)]
            # Q^T/K^T: [d-in-pair (128 = 2 heads x 64), pair, s]
            qt_sb = pers.tile([128, NP, S], bf16, tag="qt", name="qt")
            kt_sb = pers.tile([128, NP, S], bf16, tag="kt", name="kt")
            # ones-augmented V: [s-in-tile, k-tile, head, d+1] (col d = 1.0)
            v_sb = pers.tile([128, KT, hpc, D + 1], bf16, tag="v", name="v")
            mask_sb = pers.tile([128, KT], f32, tag="mask", name="mask") \
                if with_mask else None
            if with_bias:
                bq_sb = pers.tile([128, NP], f32, tag="bq", name="bq_sb")
                bk_sb = pers.tile([128, NP], f32, tag="bk", name="bk_sb")
                bvb = pers.tile([128, HD], f32, tag="bvb", name="bvb")

            # ---- input DMAs (issue order = priority) ----
            # Startup critical path carries only pair-0 weight columns and
            # X s0:256 (enough for the kt0/1 scores' K columns); the rest of
            # X streams while block 0's attention runs, pair-1 weights and
            # wv trail (first needed tens of us in).
            nc.sync.dma_start(out=wk_sbp[0][:], in_=wkp[0][:])
            nc.sync.dma_start(
                out=xts[:, :, 0:SB // 2],
                in_=xt[:, 0:SB // 2].rearrange("(c p) s -> p c s", p=128))
            nc.sync.dma_start(out=wq_sbp[0][:], in_=wqp[0][:])
            nc.sync.dma_start(
                out=xts[:, :, SB // 2:SB],
                in_=xt[:, SB // 2:SB].rearrange("(c p) s -> p c s", p=128))
            for sb in range(1, NSB):
                nc.sync.dma_start(
                    out=xts[:, :, sb * SB:(sb + 1) * SB],
                    in_=xt[:, sb * SB:(sb + 1) * SB]
                    .rearrange("(c p) s -> p c s", p=128))
            nc.sync.dma_start(out=wv_sbp[0][:], in_=wvp[0][:])
            for p in range(1, NP):
                nc.sync.dma_start(out=wk_sbp[p][:], in_=wkp[p][:])
                nc.sync.dma_start(out=wq_sbp[p][:], in_=wqp[p][:])
                nc.sync.dma_start(out=wv_sbp[p][:], in_=wvp[p][:])
            if with_mask:
                nc.sync.dma_start(
                    out=mask_sb[:], in_=msk[:].rearrange("(t p) -> p t", p=128))
            if with_bias:
                nc.sync.dma_start(
                    out=bq_sb[:], in_=bq[:].rearrange("(n p) -> p n", p=128))
                nc.sync.dma_start(
                    out=bk_sb[:], in_=bk[:].rearrange("(n p) -> p n", p=128))
                bv_ap = bv[:]
                nc.gpsimd.dma_start(
                    out=bvb[:],
                    in_=bass.AP(tensor=bv_ap.tensor, offset=bv_ap.offset,
                                ap=[[0, 128]] + list(bv_ap.ap)))

            # ones column of V_aug
            nc.vector.memset(v_sb[:, :, :, D:D + 1], 1.0)

            # PE warm-up: the cost model's p-state ramp only reaches full
            # matmul speed after ~3us of CONTINUOUS PE busy; during the
            # DMA-gated startup the PE would otherwise idle between the first
            # projection matmuls and keep resetting the ramp. Burn the DMA
            # wait on dummy matmuls over a memset tile so the real
            # projections run at full speed from the start.
            dmy = pers.tile([128, SB], bf16, tag="dmy", name="dmy")
            nc.vector.memset(dmy[:], 0.0)
            dps = pp.tile([128, SB], f32, tag="acc", name="dps")
            for _ in range(6):
                nc.tensor.matmul(dps[:], dmy[:, 0:128], dmy[:],
                                 start=True, stop=True)

            # ---- projection slices ----
            def proj_qk(which, pr, s0, s1):
                w_sb, dst = (wq_sbp, qt_sb) if which == "q" \
                    else (wk_sbp, kt_sb)
                ps = pp.tile([128, SB], f32, tag="acc", name="ps")
                for c in range(HC):
                    nc.tensor.matmul(
                        ps[:, 0:s1 - s0],
                        w_sb[pr][:, c, :],
                        xts[:, c, s0:s1],
                        start=(c == 0), stop=(c == HC - 1))
                dview = dst[:, pr, s0:s1]
                if with_bias:
                    b_sb = bq_sb if which == "q" else bk_sb
                    nc.vector.tensor_scalar_add(dview, ps[:, 0:s1 - s0],
                                                b_sb[:, pr:pr + 1])
                else:
                    nc.vector.tensor_copy(dview, ps[:, 0:s1 - s0])

            def proj_v(pr, sb, t0=0, t1=QT):
                # s-tiles of [128 s, 128 (2 heads x 64)] in one PSUM bank
                ps = pp.tile([128, QT, 128], f32, tag="acc", name="psv")
                for t in range(t0, t1):
                    st = sb * QT + t
                    for c in range(HC):
                        nc.tensor.matmul(
                            ps[:, t, :],
                            xts[:, c, st * 128:(st + 1) * 128],
                            wv_sbp[pr][:, c, :],
                            start=(c == 0), stop=(c == HC - 1))
                dview = v_sb[:, sb * QT + t0:sb * QT + t1,
                             pr * 2:pr * 2 + 2, 0:D]
                sview = ps[:, t0:t1, :].rearrange("p t (h d) -> p t h d", h=2)
                if with_bias:
                    bsl = bvb[:, pr * 128:(pr + 1) * 128] \
                        .rearrange("p (h d) -> p h d", h=2)
                    bview = bass.AP(
                        tensor=bsl.tensor, offset=bsl.offset,
                        ap=[list(bsl.ap[0]), [0, QT]]
                        + [list(a) for a in bsl.ap[1:]])
                    nc.vector.tensor_tensor(dview, sview, bview,
                                            mybir.AluOpType.add)
                else:
                    nc.vector.tensor_copy(dview, sview)

            def emit_slice(sl):
                kind = sl[0]
                if kind == "v":
                    proj_v(*sl[1:])
                    return
                if len(sl) == 4:
                    proj_qk(*sl)
                else:
                    _, pr, sb = sl
                    proj_qk(kind, pr, sb * SB, (sb + 1) * SB)

            # ---- attention: scores into a 6-bank PSUM ring, exp in
            # triples of k-tiles ----
            # Scores for consecutive (block, k-tile) steps land in a 6-slot
            # (1 bank each) PSUM ring; exp fires on up to 3 contiguous slots
            # in one [128, 1536] ACT instruction (amortizing the fixed
            # PSUM/SBUF access overhead), possibly spanning q-block
            # boundaries (exp is elementwise). E tiles persist in SBUF so
            # the ctx matmuls (emitted several blocks later,
            # software-pipelined) can run each q-tile's PSUM accumulation
            # group to completion before the next group starts -- CoreSim/HW
            # `start` marks the whole 2KB PSUM bank pending-zero, so
            # interleaved long-lived groups in one bank would clobber each
            # other.
            exp_pend = []   # [(block_idx, kt_i)] awaiting exp
            cur_ring = [None]  # triple tile being filled

            def flush_exps():
                n = len(exp_pend)
                if n == 0:
                    return
                ring = cur_ring[0]
                e = ep.tile([128, 3, SB], bf16, tag="e", name="e", bufs=42)
                if with_mask:
                    for i, (b, kt_i) in enumerate(exp_pend):
                        nc.scalar.activation(
                            e[:, i, :], ring[:, i, :], AF.Exp,
                            bias=mask_sb[:, kt_i:kt_i + 1], scale=0.125)
                else:
                    nc.scalar.activation(e[:, 0:n, :], ring[:, 0:n, :],
                                         AF.Exp, scale=0.125)
                for i, (b, kt_i) in enumerate(exp_pend):
                    es_all[b][kt_i] = (e, i)
                exp_pend.clear()
                cur_ring[0] = None

            def emit_kt(b, kt_i):
                h, qb = blocks[b]
                pr, hh = divmod(h, 2)
                if cur_ring[0] is None:
                    cur_ring[0] = psr.tile([128, 3, SB], f32, tag="ring",
                                           name="ring", bufs=2)
                nc.tensor.matmul(
                    cur_ring[0][:, len(exp_pend), :],
                    kt_sb[hh * 64:(hh + 1) * 64, pr,
                          kt_i * 128:(kt_i + 1) * 128],
                    qt_sb[hh * 64:(hh + 1) * 64, pr,
                          qb * SB:(qb + 1) * SB],
                    start=True, stop=True)
                exp_pend.append((b, kt_i))
                if len(exp_pend) == 3:
                    flush_exps()

            def ctx_mm(cps, t, kt_i, h, es, k0, k1):
                e, sub = es[kt_i]
                nc.tensor.matmul(
                    cps[:, t, :],
                    e[:, sub, t * 128:(t + 1) * 128],
                    v_sb[:, kt_i, h, :],
                    start=(kt_i == k0), stop=(kt_i == k1 - 1))

            def norm_store(cps, h, qb, ts, te):
                n = te - ts
                rcp = nrm.tile([128, QT, 1], f32, tag="rcp", name="rcp")
                nc.vector.reciprocal(out=rcp[:, ts:te, :],
                                     in_=cps[:, ts:te, D:D + 1])
                cn = nrm.tile([128, QT, D], f32, tag="cn", name="cn")
                rsl = rcp[:, ts:te, :]
                rbc = bass.AP(tensor=rsl.tensor, offset=rsl.offset,
                              ap=[list(rsl.ap[0]), list(rsl.ap[1]), [0, D]])
                nc.vector.tensor_tensor(cn[:, ts:te, :],
                                        cps[:, ts:te, 0:D], rbc,
                                        mybir.AluOpType.mult)
                q0 = qb * SB + ts * 128
                nc.sync.dma_start(
                    out=out[q0:q0 + n * 128, h * D:(h + 1) * D]
                    .rearrange("(t p) d -> p t d", p=128),
                    in_=cn[:, ts:te, :])

            def ctx_tile(use_pp, name):
                # after the projections retire, their PSUM bank serves as a
                # second ctx accumulator so back-to-back ctx blocks (the
                # pipeline-contraction doubles) don't serialize on one bank
                pool = pp if use_pp else cxp
                tag = "acc" if use_pp else "ctx"
                return pool.tile([128, QT, D + 1], f32, tag=tag, name=name)

            def ctx_block(h, qb, es, use_pp=False):
                cps = ctx_tile(use_pp, "cps")
                for t in range(QT):
                    for kt_i in range(KT):
                        ctx_mm(cps, t, kt_i, h, es, 0, KT)
                norm_store(cps, h, qb, 0, QT)

            def ctx_block_final(h, qb, es):
                # Split the k-range across two PSUM tiles so only the second
                # half's matmuls trail the last exp; combine halves on DVE.
                cA = ctx_tile(True, "cA")
                cB = ctx_tile(False, "cB")
                KH = KT - 4
                for t in range(QT):
                    for kt_i in range(KH):
                        ctx_mm(cA, t, kt_i, h, es, 0, KH)
                # evac the first half to SBUF off the critical tail (a
                # tensor_tensor may read at most one PSUM operand)
                cAs = nrm.tile([128, QT, D + 1], f32, tag="cs", name="cAs")
                nc.vector.tensor_copy(cAs[:], cA[:])
                # combine per q-tile as each B group lands, but store once:
                # four separate per-tile DMAs would serialize ~0.5us each on
                # the shared DMA device right before the closing barrier
                csf = nrm.tile([128, QT, D + 1], f32, tag="cs", name="csf")
                for t in range(QT):
                    for kt_i in range(KH, KT):
                        ctx_mm(cB, t, kt_i, h, es, KH, KT)
                    nc.vector.tensor_tensor(csf[:, t, :], cAs[:, t, :],
                                            cB[:, t, :], mybir.AluOpType.add)
                norm_store(csf, h, qb, 0, QT)

            # ---- program order / schedule ----
            # Startup: only s-block-0 projections precede the first block (the
            # PE queue is in-order; later s-blocks gate on the X DMA stream and
            # are woven in right before the kt-group that needs them).
            blocks = [(h, qb) for h in range(hpc) for qb in range(NSB)]
            es_all = [[None] * KT for _ in blocks]

            done_kp = set()

            def kps(b, *kp_list):
                for kp in kp_list:
                    if (b, kp) in done_kp:
                        continue
                    done_kp.add((b, kp))
                    emit_kt(b, 2 * kp)
                    emit_kt(b, 2 * kp + 1)

            # ---- startup: blocks 0-3 interleaved in X-arrival order ----
            # The X^T stream (4 s-blocks, ~3us each) gates both the K slices
            # (k-tiles of later kt-pairs) and the Q slices (later q-blocks).
            # Processing (block, k-tile) pairs diagonally by availability
            # keeps ACT busy from ~7us with no X-paced stalls; a plain
            # block-major order would idle ACT until the last s-block lands.
            proj_qk("k", 0, 0, 256)
            proj_qk("q", 0, 0, 256)
            proj_qk("q", 0, 256, SB)
            kps(0, 0)
            flush_exps()  # 2-kt first group: ACT starts before X s256:512
            proj_qk("k", 0, 256, 512)
            kps(0, 1)
            kps(4, 0, 1)
            flush_exps()  # sb0 boundary: don't straddle into sb1-gated kts
            proj_qk("k", 0, 512, 768)
            kps(0, 2)
            proj_qk("k", 0, 768, 1024)
            kps(0, 3)
            emit_slice(("q", 0, 1))
            kps(1, 0, 1)
            kps(4, 2, 3)
            kps(5, 0, 1)
            kps(5, 2, 3)
            flush_exps()  # sb1 boundary
            kps(1, 2, 3)  # sb1-ready filler while the X sb2 DMA lands
            proj_qk("k", 0, 1024, 1280)
            kps(0, 4)
            proj_qk("k", 0, 1280, 1536)
            kps(0, 5)
            emit_slice(("q", 0, 2))
            kps(1, 2, 3)
            kps(2, 0, 1)
            kps(4, 4, 5)
            flush_exps()  # sb2 boundary
            proj_qk("k", 0, 1536, 1792)
            kps(0, 6)
            proj_qk("k", 0, 1792, 2048)
            kps(0, 7)
            emit_slice(("q", 0, 3))
            kps(1, 4, 5)
            kps(2, 2, 3)
            kps(3, 0, 1)
            emit_slice(("v", 0, 0))
            kps(1, 6, 7)
            kps(2, 4, 5)
            emit_slice(("v", 0, 1))
            kps(3, 2, 3)
            emit_slice(("v", 0, 2))
            kps(2, 6, 7)
            kps(3, 4, 5)
            emit_slice(("v", 0, 3))
            kps(3, 6, 7)

            # ---- steady state: scores(i) + woven pair-1 projections, with
            # ctx(i) trailing (variable depth, contracting to 1 at the end
            # so only one ctx block trails the last exp) ----
            hooks = {
                5: {0: [("k", 1, 0, 256)], 1: [("k", 1, 256, 512)],
                    4: [("k", 1, 512, 768)], 5: [("k", 1, 768, 1024)]},
                6: {0: [("k", 1, 1024, 1280)], 1: [("k", 1, 1280, 1536)],
                    4: [("k", 1, 1536, 1792)], 5: [("k", 1, 1792, 2048)]},
                7: {0: [("q", 1, 0, 256)], 1: [("q", 1, 256, 512)],
                    4: [("v", 1, 0, 0, 2)], 5: [("v", 1, 0, 2, 4)]},
                8: {0: [("q", 1, 512, 768)], 1: [("q", 1, 768, 1024)],
                    4: [("v", 1, 1, 0, 2)], 5: [("v", 1, 1, 2, 4)]},
                9: {0: [("q", 1, 1024, 1280)], 1: [("q", 1, 1280, 1536)],
                    4: [("v", 1, 2, 0, 2)], 5: [("v", 1, 2, 2, 4)]},
                10: {0: [("q", 1, 1536, 1792)], 1: [("q", 1, 1792, 2048)]},
                11: {0: [("v", 1, 3, 0, 2)], 1: [("v", 1, 3, 2, 4)]},
            }
            sched = [-1, 4, -1, 5, -1, 6, -1, 7, -1, 8, -1, 9, -1,
                     10, -1, 11, -1, 12, -1, -1, 13, -1, -1, 14, -1,
                     15, -1, -1]
            # pending ctx blocks are emitted one q-tile group (16 matmuls)
            # at a time between kt-pairs, so the PE never inserts a long
            # scores gap that would drain ACT's one-triple backlog
            pend_ctx = []

            def pump_ctx():
                if not pend_ctx:
                    return
                st = pend_ctx[0]
                t = st["t"]
                for kt_i in range(KT):
                    ctx_mm(st["tile"], t, kt_i, st["h"], st["es"], 0, KT)
                st["t"] += 1
                if st["t"] == QT:
                    norm_store(st["tile"], st["h"], st["qb"], 0, QT)
                    pend_ctx.pop(0)

            nxt_ctx = 0
            for item in sched:
                if item >= 0:
                    bhooks = hooks.get(item, {})
                    for kp in range(KP):
                        for sl in bhooks.get(kp, []):
                            emit_slice(sl)
                        kps(item, kp)
                        pump_ctx()
                else:
                    h, qb = blocks[nxt_ctx]
                    es = es_all[nxt_ctx]
                    if nxt_ctx == len(blocks) - 1:
                        flush_exps()
                        while pend_ctx:
                            pump_ctx()
                        ctx_block_final(h, qb, es)
                    else:
                        use_pp = nxt_ctx >= 9 and nxt_ctx % 2 == 1
                        pend_ctx.append(
                            {"h": h, "qb": qb, "es": es, "t": 0,
                             "tile": ctx_tile(use_pp, f"cps{nxt_ctx}")})
                    nxt_ctx += 1

    _split_multi_waits(nc, mybir)
    return nc


def _split_multi_waits(nc, mybir):
    """This walrus build packs at most ONE sync-wait into an instruction
    (setupSyncWait<...CTRL_NO_STRUCT> rejects Tile's multi-wait drains), so
    hoist all but the last wait of every instruction onto dedicated
    single-wait InstEventSemaphore carriers inserted just before it on the
    same engine. Waits are AND-conditions; a sequential chain on the same
    sequencer is equivalent."""
    n = 0
    for f in nc.m.functions:
        for b in f.blocks:
            ins_list = list(b.instructions)
            out_list = []
            changed = False
            for ins in ins_list:
                si = ins.sync_info
                if si and si.on_wait and len(si.on_wait) > 1:
                    waits = list(si.on_wait)
                    for w in waits[:-1]:
                        carrier = mybir.InstEventSemaphore(
                            name=f"waitsplit-{n}", ins=[], outs=[])
                        n += 1
                        carrier.engine = ins.engine
                        carrier.sync_info = mybir.SyncInfo(on_wait=[w],
                                                           on_update=[])
                        nc.register_instruction(carrier)
                        out_list.append(carrier)
                    si.on_wait = waits[-1:]
                    changed = True
                out_list.append(ins)
            if changed:
                b.instructions = out_list


def _shard_inputs(hs, am, Wq, bq, Wk, bk, Wv, bv, with_mask, with_bias, hpc):
    import ml_dtypes
    bf16 = ml_dtypes.bfloat16
    hd = hpc * DHEAD
    in_maps = []
    for c in range(NCORES):
        b = c // CORES_PER_BATCH
        g = c % CORES_PER_BATCH
        cols = slice(g * hd, (g + 1) * hd)
        m = {"xt": np.ascontiguousarray(hs[b].T.astype(bf16))}
        # weights in the on-chip layout, one tensor per (matrix, head-pair):
        # [partition (h%128), chunk (h//128), 128 out-cols]
        for wname, W in (("wq", Wq), ("wk", Wk), ("wv", Wv)):
            for p in range(hd // 128):
                cols_p = slice(g * hd + p * 128, g * hd + (p + 1) * 128)
                m[f"{wname}{p}"] = np.ascontiguousarray(
                    W[:, cols_p].astype(bf16).reshape(-1, 128, 128)
                    .transpose(1, 0, 2))
        if with_bias:
            m["bq"] = np.ascontiguousarray(bq[cols])
            m["bk"] = np.ascontiguousarray(bk[cols])
            m["bv"] = np.ascontiguousarray(bv[cols])
        if with_mask:
            m["mask"] = np.ascontiguousarray(am[b, 0, 0, :])
        in_maps.append(m)
    return in_maps


def kernel(hidden_states, attention_mask, Wq, bq, Wk, bk, Wv, bv):
    global LAST_RESULT
    hs = np.asarray(hidden_states, dtype=np.float32)
    am = np.asarray(attention_mask, dtype=np.float32)
    Wq = np.asarray(Wq, dtype=np.float32)
    Wk = np.asarray(Wk, dtype=np.float32)
    Wv = np.asarray(Wv, dtype=np.float32)
    bq = np.asarray(bq, dtype=np.float32)
    bk = np.asarray(bk, dtype=np.float32)
    bv = np.asarray(bv, dtype=np.float32)

    B, S, H = hs.shape
    assert (B, S, H) == (B_FULL, S_FULL, H_FULL), "kernel is shape-specialized"
    with_mask = bool(np.any(am))
    with_bias = bool(np.any(bq) or np.any(bk) or np.any(bv))

    nc = _build(S, H, HEADS_PER_CORE, with_mask, with_bias)

    from concourse.bass_utils import run_bass_kernel_spmd
    in_maps = _shard_inputs(hs, am, Wq, bq, Wk, bk, Wv, bv, with_mask,
                            with_bias, HEADS_PER_CORE)
    # NTFF tracing is unavailable under this axon client (antenv.axon_hooks
    # is absent); make sure an inherited BASS_TRACE can't divert the run
    # into that path.
    import os
    prev = os.environ.get("BASS_NEVER_TRACE")
    os.environ["BASS_NEVER_TRACE"] = "1"
    try:
        res = run_bass_kernel_spmd(nc, in_maps, core_ids=list(range(NCORES)))
    finally:
        if prev is None:
            os.environ.pop("BASS_NEVER_TRACE", None)
        else:
            os.environ["BASS_NEVER_TRACE"] = prev
    LAST_RESULT = res

    hd = HEADS_PER_CORE * DHEAD
    outp = np.empty((B, S, H), dtype=np.float32)
    for c in range(NCORES):
        b = c // CORES_PER_BATCH
        g = c % CORES_PER_BATCH
        outp[b, :, g * hd:(g + 1) * hd] = res.results[c]["out"]
    return outp


# revision 57
# speedup vs baseline: 1.0022x; 1.0022x over previous
"""TRN2 Bass/Tile kernel: BERT self-attention (B=2, S=2048, H=1024, 16 heads, d=64).

Sharding (host side, all 8 cores run one SPMD NEFF):
  core c: batch b = c // 4, head group g = c % 4 (heads 4g..4g+3 = weight cols
  256g..256g+256). Each core receives X^T [H, S] for its batch (host transpose,
  cast to bf16), plus its weight columns pre-rearranged per head-pair into the
  on-chip [128, chunk, 128] layout (contiguous 2KB DMA rows), and returns its
  [S, 256] fp32 slice of the output in natural orientation.

Device algorithm (per core), bf16 matmul inputs / fp32 PSUM accumulation:
  1. Projections: Q^T/K^T in [d-pair(128), pair, s] layout, V in natural
     [s, kt, head, d+1] layout with a constant-1 column (ones-augmented V).
     All matmuls stream >=128 output rows in bf16 (1 PE cycle/row). PSUM is
     evacuated to bf16 SBUF by the DVE engine so the ACT engine stays
     dedicated to exp (the global bottleneck: S*S*4heads/128 rows = ~110us
     of exp per core; everything else hides behind it).
  2. Scores: scoresT[k, q] = K Q^T per (q-block 512, k-tile 128), bf16,
     512 free rows/matmul, landing in a ring of [128, 3, 512] PSUM triples
     (2 bufs = 6 banks). exp fires on ACT over whole triples ([128, 1536]
     per instruction, possibly spanning q-block boundaries -- exp is
     elementwise) to amortize the fixed PSUM/SBUF access overhead, writing
     persistent bf16 E tiles.
  3. ctx[q, d+1] += E^T-stationary x V_aug accumulated over k in PSUM
     (col d = softmax denominator via the ones column). Each q-tile's
     accumulation group runs to completion before the next group starts in
     the same bank: `start` marks the whole 2KB PSUM bank pending-zero, so
     interleaved long-lived groups in one bank clobber each other.
  4. Normalize on DVE: reciprocal of the denominator column + broadcast
     multiply, then DMA the [q, 4x64] block to the fp32 output.

  Scheduling (the PE queue is in-order, so emission order is the schedule):
  - Startup processes (block, k-tile) pairs of the first q-blocks of BOTH
    heads of pair 0 diagonally in X-arrival order, with K/Q projection
    slices at 256-wide granularity woven in right before the k-tiles that
    need them; h1's q-block reuses the pair-0 projections so the exp stream
    starts at ~7us and stays fed while X streams in.
  - Steady state weaves the pair-1 projections into later blocks, and
    trailing ctx blocks are pumped one q-tile group (16 matmuls) at a time
    between kt-pairs so the PE never inserts a scores gap longer than ACT's
    one-triple backlog.
  - The final ctx block splits its k-range 12/4 across two PSUM banks so
    only the last 4 k-tiles' matmuls (plus a DVE combine) trail the last
    exp.
  - PE warm-up dummy matmuls at t=0 keep the p-state ramp from resetting
    during the DMA-gated start.

  Softmax skips the row-max subtraction (scores ~ N(0,1) after the 1/8
  scale; exp cannot overflow) and defers normalization to the ctx output.
  bf16 end-to-end relative error is ~5e-3 (well under the 2e-2 gate).

  The spec pins biases and attention_mask to zeros; nonzero values get a
  generic (slightly slower) variant selected at build time, zeros skip the
  work entirely.

  _split_multi_waits: this walrus build packs at most one sync-wait per
  instruction, so Tile's multi-wait instructions get their extra waits
  hoisted onto single-wait InstEventSemaphore carriers.
"""

import functools
import numpy as np

B_FULL = 2
S_FULL = 2048
H_FULL = 1024
NHEADS = 16
DHEAD = 64
NCORES = 8
CORES_PER_BATCH = 4
HEADS_PER_CORE = NHEADS // CORES_PER_BATCH  # 4

# Stash of the last BassKernelResults (test harness reads exec_time_ns off it).
LAST_RESULT = None


@functools.lru_cache(maxsize=None)
def _build(S, H, hpc, with_mask, with_bias):
    import concourse.bass as bass
    import concourse.tile as tile
    import concourse.mybir as mybir

    f32 = mybir.dt.float32
    bf16 = mybir.dt.bfloat16
    AF = mybir.ActivationFunctionType
    D = DHEAD
    HD = hpc * D            # output columns per core (256)
    NP = hpc // 2           # head pairs per core (2)
    HC = H // 128           # contraction chunks (8)
    SB = 512                # s-block for projections / q-block for attention
    NSB = S // SB           # 4
    KT = S // 128           # k-tiles (16)
    KP = KT // 2            # kt-pairs per attention block (8)
    QT = SB // 128          # q-tiles per q-block (4)
    assert S % SB == 0 and H % 128 == 0 and hpc % 2 == 0

    nc = bass.Bass()
    xt = nc.dram_tensor("xt", [H, S], bf16, kind="ExternalInput")
    # weights arrive host-rearranged to the on-chip layout, one tensor per
    # (matrix, head-pair): [partition(h%128), chunk(h//128), 128 out-cols]
    # so each pair's DMA moves contiguous 2KB rows (no small-row penalty)
    wqp = [nc.dram_tensor(f"wq{p}", [128, HC, 128], bf16,
                          kind="ExternalInput") for p in range(NP)]
    wkp = [nc.dram_tensor(f"wk{p}", [128, HC, 128], bf16,
                          kind="ExternalInput") for p in range(NP)]
    wvp = [nc.dram_tensor(f"wv{p}", [128, HC, 128], bf16,
                          kind="ExternalInput") for p in range(NP)]
    if with_bias:
        bq = nc.dram_tensor("bq", [HD], f32, kind="ExternalInput")
        bk = nc.dram_tensor("bk", [HD], f32, kind="ExternalInput")
        bv = nc.dram_tensor("bv", [HD], f32, kind="ExternalInput")
    msk = nc.dram_tensor("mask", [S], f32, kind="ExternalInput") if with_mask else None
    out = nc.dram_tensor("out", [S, HD], f32, kind="ExternalOutput")

    with tile.TileContext(nc) as tc:
        with tc.tile_pool(name="pers", bufs=1) as pers, \
             tc.tile_pool(name="pp", bufs=1, space="PSUM") as pp, \
             tc.tile_pool(name="psr", bufs=1, space="PSUM") as psr, \
             tc.tile_pool(name="cxp", bufs=1, space="PSUM") as cxp, \
             tc.tile_pool(name="ep", bufs=4) as ep, \
             tc.tile_pool(name="nrm", bufs=3) as nrm:
            # ---- persistent SBUF ----
            xts = pers.tile([128, HC, S], bf16, tag="xts", name="xts")
            wq_sbp = [pers.tile([128, HC, 128], bf16, tag=f"wq{p}",
                                name=f"wq_sb{p}") for p in range(NP)]
            wk_sbp = [pers.tile([128, HC, 128], bf16, tag=f"wk{p}",
                                name=f"wk_sb{p}") for p in range(NP)]
            wv_sbp = [pers.tile([128, HC, 128], bf16, tag=f"wv{p}",
                                name=f"wv_sb{p}") for p in range(NP)]
            # Q^T/K^T: [d-in-pair (128 = 2 heads x 64), pair, s]
            qt_sb = pers.tile([128, NP, S], bf16, tag="qt", name="qt")
            kt_sb = pers.tile([128, NP, S], bf16, tag="kt", name="kt")
            # ones-augmented V: [s-in-tile, k-tile, head, d+1] (col d = 1.0)
            v_sb = pers.tile([128, KT, hpc, D + 1], bf16, tag="v", name="v")
            mask_sb = pers.tile([128, KT], f32, tag="mask", name="mask") \
                if with_mask else None
            if with_bias:
                bq_sb = pers.tile([128, NP], f32, tag="bq", name="bq_sb")
                bk_sb = pers.tile([128, NP], f32, tag="bk", name="bk_sb")
                bvb = pers.tile([128, HD], f32, tag="bvb", name="bvb")

            # ---- input DMAs (issue order = priority) ----
            # Startup critical path carries only pair-0 weight columns and
            # X s0:256 (enough for the kt0/1 scores' K columns); the rest of
            # X streams while block 0's attention runs, pair-1 weights and
            # wv trail (first needed tens of us in).
            nc.sync.dma_start(out=wk_sbp[0][:], in_=wkp[0][:])
            nc.sync.dma_start(
                out=xts[:, :, 0:SB // 2],
                in_=xt[:, 0:SB // 2].rearrange("(c p) s -> p c s", p=128))
            nc.sync.dma_start(out=wq_sbp[0][:], in_=wqp[0][:])
            nc.sync.dma_start(
                out=xts[:, :, SB // 2:SB],
                in_=xt[:, SB // 2:SB].rearrange("(c p) s -> p c s", p=128))
            # sb1 split in two: its first half gates the post-sb0 K
            # projection chain, so land it ~1.5us earlier
            nc.sync.dma_start(
                out=xts[:, :, SB:SB + 256],
                in_=xt[:, SB:SB + 256].rearrange("(c p) s -> p c s", p=128))
            nc.sync.dma_start(
                out=xts[:, :, SB + 256:2 * SB],
                in_=xt[:, SB + 256:2 * SB]
                .rearrange("(c p) s -> p c s", p=128))
            for sb in range(2, NSB):
                nc.sync.dma_start(
                    out=xts[:, :, sb * SB:(sb + 1) * SB],
                    in_=xt[:, sb * SB:(sb + 1) * SB]
                    .rearrange("(c p) s -> p c s", p=128))
            nc.sync.dma_start(out=wv_sbp[0][:], in_=wvp[0][:])
            for p in range(1, NP):
                nc.sync.dma_start(out=wk_sbp[p][:], in_=wkp[p][:])
                nc.sync.dma_start(out=wq_sbp[p][:], in_=wqp[p][:])
                nc.sync.dma_start(out=wv_sbp[p][:], in_=wvp[p][:])
            if with_mask:
                nc.sync.dma_start(
                    out=mask_sb[:], in_=msk[:].rearrange("(t p) -> p t", p=128))
            if with_bias:
                nc.sync.dma_start(
                    out=bq_sb[:], in_=bq[:].rearrange("(n p) -> p n", p=128))
                nc.sync.dma_start(
                    out=bk_sb[:], in_=bk[:].rearrange("(n p) -> p n", p=128))
                bv_ap = bv[:]
                nc.gpsimd.dma_start(
                    out=bvb[:],
                    in_=bass.AP(tensor=bv_ap.tensor, offset=bv_ap.offset,
                                ap=[[0, 128]] + list(bv_ap.ap)))

            # ones column of V_aug
            nc.vector.memset(v_sb[:, :, :, D:D + 1], 1.0)

            # PE warm-up: the cost model's p-state ramp only reaches full
            # matmul speed after ~3us of CONTINUOUS PE busy; during the
            # DMA-gated startup the PE would otherwise idle between the first
            # projection matmuls and keep resetting the ramp. Burn the DMA
            # wait on dummy matmuls over a memset tile so the real
            # projections run at full speed from the start.
            dmy = pers.tile([128, SB], bf16, tag="dmy", name="dmy")
            nc.vector.memset(dmy[:], 0.0)
            dps = pp.tile([128, SB], f32, tag="acc", name="dps")
            for _ in range(6):
                nc.tensor.matmul(dps[:], dmy[:, 0:128], dmy[:],
                                 start=True, stop=True)

            # ---- projection slices ----
            def proj_qk(which, pr, s0, s1):
                w_sb, dst = (wq_sbp, qt_sb) if which == "q" \
                    else (wk_sbp, kt_sb)
                ps = pp.tile([128, SB], f32, tag="acc", name="ps")
                for c in range(HC):
                    nc.tensor.matmul(
                        ps[:, 0:s1 - s0],
                        w_sb[pr][:, c, :],
                        xts[:, c, s0:s1],
                        start=(c == 0), stop=(c == HC - 1))
                dview = dst[:, pr, s0:s1]
                if with_bias:
                    b_sb = bq_sb if which == "q" else bk_sb
                    nc.vector.tensor_scalar_add(dview, ps[:, 0:s1 - s0],
                                                b_sb[:, pr:pr + 1])
                else:
                    nc.vector.tensor_copy(dview, ps[:, 0:s1 - s0])

            def proj_v(pr, sb, t0=0, t1=QT):
                # s-tiles of [128 s, 128 (2 heads x 64)] in one PSUM bank
                ps = pp.tile([128, QT, 128], f32, tag="acc", name="psv")
                for t in range(t0, t1):
                    st = sb * QT + t
                    for c in range(HC):
                        nc.tensor.matmul(
                            ps[:, t, :],
                            xts[:, c, st * 128:(st + 1) * 128],
                            wv_sbp[pr][:, c, :],
                            start=(c == 0), stop=(c == HC - 1))
                dview = v_sb[:, sb * QT + t0:sb * QT + t1,
                             pr * 2:pr * 2 + 2, 0:D]
                sview = ps[:, t0:t1, :].rearrange("p t (h d) -> p t h d", h=2)
                if with_bias:
                    bsl = bvb[:, pr * 128:(pr + 1) * 128] \
                        .rearrange("p (h d) -> p h d", h=2)
                    bview = bass.AP(
                        tensor=bsl.tensor, offset=bsl.offset,
                        ap=[list(bsl.ap[0]), [0, QT]]
                        + [list(a) for a in bsl.ap[1:]])
                    nc.vector.tensor_tensor(dview, sview, bview,
                                            mybir.AluOpType.add)
                else:
                    nc.vector.tensor_copy(dview, sview)

            def emit_slice(sl):
                kind = sl[0]
                if kind == "v":
                    proj_v(*sl[1:])
                    return
                if len(sl) == 4:
                    proj_qk(*sl)
                else:
                    _, pr, sb = sl
                    proj_qk(kind, pr, sb * SB, (sb + 1) * SB)

            # ---- attention: scores into a 6-bank PSUM ring, exp in
            # triples of k-tiles ----
            # Scores for consecutive (block, k-tile) steps land in a 6-slot
            # (1 bank each) PSUM ring; exp fires on up to 3 contiguous slots
            # in one [128, 1536] ACT instruction (amortizing the fixed
            # PSUM/SBUF access overhead), possibly spanning q-block
            # boundaries (exp is elementwise). E tiles persist in SBUF so
            # the ctx matmuls (emitted several blocks later,
            # software-pipelined) can run each q-tile's PSUM accumulation
            # group to completion before the next group starts -- CoreSim/HW
            # `start` marks the whole 2KB PSUM bank pending-zero, so
            # interleaved long-lived groups in one bank would clobber each
            # other.
            exp_pend = []   # [(block_idx, kt_i)] awaiting exp
            cur_ring = [None]  # triple tile being filled

            def flush_exps():
                n = len(exp_pend)
                if n == 0:
                    return
                ring = cur_ring[0]
                e = ep.tile([128, 3, SB], bf16, tag="e", name="e", bufs=42)
                if with_mask:
                    for i, (b, kt_i) in enumerate(exp_pend):
                        nc.scalar.activation(
                            e[:, i, :], ring[:, i, :], AF.Exp,
                            bias=mask_sb[:, kt_i:kt_i + 1], scale=0.125)
                else:
                    nc.scalar.activation(e[:, 0:n, :], ring[:, 0:n, :],
                                         AF.Exp, scale=0.125)
                for i, (b, kt_i) in enumerate(exp_pend):
                    es_all[b][kt_i] = (e, i)
                exp_pend.clear()
                cur_ring[0] = None

            def emit_kt(b, kt_i):
                h, qb = blocks[b]
                pr, hh = divmod(h, 2)
                if cur_ring[0] is None:
                    cur_ring[0] = psr.tile([128, 3, SB], f32, tag="ring",
                                           name="ring", bufs=2)
                nc.tensor.matmul(
                    cur_ring[0][:, len(exp_pend), :],
                    kt_sb[hh * 64:(hh + 1) * 64, pr,
                          kt_i * 128:(kt_i + 1) * 128],
                    qt_sb[hh * 64:(hh + 1) * 64, pr,
                          qb * SB:(qb + 1) * SB],
                    start=True, stop=True)
                exp_pend.append((b, kt_i))
                if len(exp_pend) == 3:
                    flush_exps()

            def ctx_mm(cps, t, kt_i, h, es, k0, k1):
                e, sub = es[kt_i]
                nc.tensor.matmul(
                    cps[:, t, :],
                    e[:, sub, t * 128:(t + 1) * 128],
                    v_sb[:, kt_i, h, :],
                    start=(kt_i == k0), stop=(kt_i == k1 - 1))

            def norm_store(cps, h, qb, ts, te):
                n = te - ts
                rcp = nrm.tile([128, QT, 1], f32, tag="rcp", name="rcp")
                nc.vector.reciprocal(out=rcp[:, ts:te, :],
                                     in_=cps[:, ts:te, D:D + 1])
                cn = nrm.tile([128, QT, D], f32, tag="cn", name="cn")
                rsl = rcp[:, ts:te, :]
                rbc = bass.AP(tensor=rsl.tensor, offset=rsl.offset,
                              ap=[list(rsl.ap[0]), list(rsl.ap[1]), [0, D]])
                nc.vector.tensor_tensor(cn[:, ts:te, :],
                                        cps[:, ts:te, 0:D], rbc,
                                        mybir.AluOpType.mult)
                q0 = qb * SB + ts * 128
                nc.sync.dma_start(
                    out=out[q0:q0 + n * 128, h * D:(h + 1) * D]
                    .rearrange("(t p) d -> p t d", p=128),
                    in_=cn[:, ts:te, :])

            def ctx_tile(use_pp, name):
                # after the projections retire, their PSUM bank serves as a
                # second ctx accumulator so back-to-back ctx blocks (the
                # pipeline-contraction doubles) don't serialize on one bank
                pool = pp if use_pp else cxp
                tag = "acc" if use_pp else "ctx"
                return pool.tile([128, QT, D + 1], f32, tag=tag, name=name)

            def ctx_block(h, qb, es, use_pp=False):
                cps = ctx_tile(use_pp, "cps")
                for t in range(QT):
                    for kt_i in range(KT):
                        ctx_mm(cps, t, kt_i, h, es, 0, KT)
                norm_store(cps, h, qb, 0, QT)

            def ctx_block_final(h, qb, es):
                # Split the k-range across two PSUM tiles so only the second
                # half's matmuls trail the last exp; combine halves on DVE.
                cA = ctx_tile(True, "cA")
                cB = ctx_tile(False, "cB")
                KH = KT - 4
                for t in range(QT):
                    for kt_i in range(KH):
                        ctx_mm(cA, t, kt_i, h, es, 0, KH)
                # evac the first half to SBUF off the critical tail (a
                # tensor_tensor may read at most one PSUM operand)
                cAs = nrm.tile([128, QT, D + 1], f32, tag="cs", name="cAs")
                nc.vector.tensor_copy(cAs[:], cA[:])
                for t in range(QT):
                    for kt_i in range(KH, KT):
                        ctx_mm(cB, t, kt_i, h, es, KH, KT)
                    cs = nrm.tile([128, QT, D + 1], f32, tag="cs", name="cs")
                    nc.vector.tensor_tensor(cs[:, t, :], cAs[:, t, :],
                                            cB[:, t, :], mybir.AluOpType.add)
                    norm_store(cs, h, qb, t, t + 1)

            # ---- program order / schedule ----
            # Startup: only s-block-0 projections precede the first block (the
            # PE queue is in-order; later s-blocks gate on the X DMA stream and
            # are woven in right before the kt-group that needs them).
            blocks = [(h, qb) for h in range(hpc) for qb in range(NSB)]
            es_all = [[None] * KT for _ in blocks]

            done_kp = set()

            def kps(b, *kp_list):
                for kp in kp_list:
                    if (b, kp) in done_kp:
                        continue
                    done_kp.add((b, kp))
                    emit_kt(b, 2 * kp)
                    emit_kt(b, 2 * kp + 1)

            # ---- startup: blocks 0-3 interleaved in X-arrival order ----
            # The X^T stream (4 s-blocks, ~3us each) gates both the K slices
            # (k-tiles of later kt-pairs) and the Q slices (later q-blocks).
            # Processing (block, k-tile) pairs diagonally by availability
            # keeps ACT busy from ~7us with no X-paced stalls; a plain
            # block-major order would idle ACT until the last s-block lands.
            proj_qk("k", 0, 0, 256)
            proj_qk("q", 0, 0, 256)
            proj_qk("q", 0, 256, SB)
            kps(0, 0)
            flush_exps()  # 2-kt first group: ACT starts before X s256:512
            proj_qk("k", 0, 256, 512)
            kps(0, 1)
            kps(4, 0, 1)
            flush_exps()  # sb0 boundary: don't straddle into sb1-gated kts
            proj_qk("k", 0, 512, 768)
            kps(0, 2)
            proj_qk("k", 0, 768, 1024)
            kps(0, 3)
            emit_slice(("q", 0, 1))
            kps(1, 0, 1)
            kps(4, 2, 3)
            kps(5, 0, 1)
            kps(5, 2, 3)
            flush_exps()  # sb1 boundary
            kps(1, 2, 3)  # sb1-ready filler while the X sb2 DMA lands
            proj_qk("k", 0, 1024, 1280)
            kps(0, 4)
            proj_qk("k", 0, 1280, 1536)
            kps(0, 5)
            emit_slice(("q", 0, 2))
            kps(1, 2, 3)
            kps(2, 0, 1)
            kps(4, 4, 5)
            flush_exps()  # sb2 boundary
            proj_qk("k", 0, 1536, 1792)
            kps(0, 6)
            proj_qk("k", 0, 1792, 2048)
            kps(0, 7)
            emit_slice(("q", 0, 3))
            kps(1, 4, 5)
            kps(2, 2, 3)
            kps(3, 0, 1)
            emit_slice(("v", 0, 0))
            kps(1, 6, 7)
            kps(2, 4, 5)
            emit_slice(("v", 0, 1))
            kps(3, 2, 3)
            emit_slice(("v", 0, 2))
            kps(2, 6, 7)
            kps(3, 4, 5)
            emit_slice(("v", 0, 3))
            kps(3, 6, 7)

            # ---- steady state: scores(i) + woven pair-1 projections, with
            # ctx(i) trailing (variable depth, contracting to 1 at the end
            # so only one ctx block trails the last exp) ----
            hooks = {
                5: {0: [("k", 1, 0, 256)], 1: [("k", 1, 256, 512)],
                    4: [("k", 1, 512, 768)], 5: [("k", 1, 768, 1024)]},
                6: {0: [("k", 1, 1024, 1280)], 1: [("k", 1, 1280, 1536)],
                    4: [("k", 1, 1536, 1792)], 5: [("k", 1, 1792, 2048)]},
                7: {0: [("q", 1, 0, 256)], 1: [("q", 1, 256, 512)],
                    4: [("v", 1, 0, 0, 2)], 5: [("v", 1, 0, 2, 4)]},
                8: {0: [("q", 1, 512, 768)], 1: [("q", 1, 768, 1024)],
                    4: [("v", 1, 1, 0, 2)], 5: [("v", 1, 1, 2, 4)]},
                9: {0: [("q", 1, 1024, 1280)], 1: [("q", 1, 1280, 1536)],
                    4: [("v", 1, 2, 0, 2)], 5: [("v", 1, 2, 2, 4)]},
                10: {0: [("q", 1, 1536, 1792)], 1: [("q", 1, 1792, 2048)]},
                11: {0: [("v", 1, 3, 0, 2)], 1: [("v", 1, 3, 2, 4)]},
            }
            sched = [-1, 4, -1, 5, -1, 6, -1, 7, -1, 8, -1, 9, -1,
                     10, -1, 11, -1, 12, -1, -1, 13, -1, -1, 14, -1,
                     15, -1, -1]
            # pending ctx blocks are emitted one q-tile group (16 matmuls)
            # at a time between kt-pairs, so the PE never inserts a long
            # scores gap that would drain ACT's one-triple backlog
            pend_ctx = []

            def pump_ctx():
                if not pend_ctx:
                    return
                st = pend_ctx[0]
                t = st["t"]
                for kt_i in range(KT):
                    ctx_mm(st["tile"], t, kt_i, st["h"], st["es"], 0, KT)
                st["t"] += 1
                if st["t"] == QT:
                    norm_store(st["tile"], st["h"], st["qb"], 0, QT)
                    pend_ctx.pop(0)

            nxt_ctx = 0
            for item in sched:
                if item >= 0:
                    bhooks = hooks.get(item, {})
                    for kp in range(KP):
                        for sl in bhooks.get(kp, []):
                            emit_slice(sl)
                        kps(item, kp)
                        pump_ctx()
                else:
                    h, qb = blocks[nxt_ctx]
                    es = es_all[nxt_ctx]
                    if nxt_ctx == len(blocks) - 1:
                        flush_exps()
                        while pend_ctx:
                            pump_ctx()
                        ctx_block_final(h, qb, es)
                    else:
                        use_pp = nxt_ctx >= 9 and nxt_ctx % 2 == 1
                        pend_ctx.append(
                            {"h": h, "qb": qb, "es": es, "t": 0,
                             "tile": ctx_tile(use_pp, f"cps{nxt_ctx}")})
                    nxt_ctx += 1

    _split_multi_waits(nc, mybir)
    return nc


def _split_multi_waits(nc, mybir):
    """This walrus build packs at most ONE sync-wait into an instruction
    (setupSyncWait<...CTRL_NO_STRUCT> rejects Tile's multi-wait drains), so
    hoist all but the last wait of every instruction onto dedicated
    single-wait InstEventSemaphore carriers inserted just before it on the
    same engine. Waits are AND-conditions; a sequential chain on the same
    sequencer is equivalent."""
    n = 0
    for f in nc.m.functions:
        for b in f.blocks:
            ins_list = list(b.instructions)
            out_list = []
            changed = False
            for ins in ins_list:
                si = ins.sync_info
                if si and si.on_wait and len(si.on_wait) > 1:
                    waits = list(si.on_wait)
                    for w in waits[:-1]:
                        carrier = mybir.InstEventSemaphore(
                            name=f"waitsplit-{n}", ins=[], outs=[])
                        n += 1
                        carrier.engine = ins.engine
                        carrier.sync_info = mybir.SyncInfo(on_wait=[w],
                                                           on_update=[])
                        nc.register_instruction(carrier)
                        out_list.append(carrier)
                    si.on_wait = waits[-1:]
                    changed = True
                out_list.append(ins)
            if changed:
                b.instructions = out_list


def _shard_inputs(hs, am, Wq, bq, Wk, bk, Wv, bv, with_mask, with_bias, hpc):
    import ml_dtypes
    bf16 = ml_dtypes.bfloat16
    hd = hpc * DHEAD
    in_maps = []
    for c in range(NCORES):
        b = c // CORES_PER_BATCH
        g = c % CORES_PER_BATCH
        cols = slice(g * hd, (g + 1) * hd)
        m = {"xt": np.ascontiguousarray(hs[b].T.astype(bf16))}
        # weights in the on-chip layout, one tensor per (matrix, head-pair):
        # [partition (h%128), chunk (h//128), 128 out-cols]
        for wname, W in (("wq", Wq), ("wk", Wk), ("wv", Wv)):
            for p in range(hd // 128):
                cols_p = slice(g * hd + p * 128, g * hd + (p + 1) * 128)
                m[f"{wname}{p}"] = np.ascontiguousarray(
                    W[:, cols_p].astype(bf16).reshape(-1, 128, 128)
                    .transpose(1, 0, 2))
        if with_bias:
            m["bq"] = np.ascontiguousarray(bq[cols])
            m["bk"] = np.ascontiguousarray(bk[cols])
            m["bv"] = np.ascontiguousarray(bv[cols])
        if with_mask:
            m["mask"] = np.ascontiguousarray(am[b, 0, 0, :])
        in_maps.append(m)
    return in_maps


def kernel(hidden_states, attention_mask, Wq, bq, Wk, bk, Wv, bv):
    global LAST_RESULT
    hs = np.asarray(hidden_states, dtype=np.float32)
    am = np.asarray(attention_mask, dtype=np.float32)
    Wq = np.asarray(Wq, dtype=np.float32)
    Wk = np.asarray(Wk, dtype=np.float32)
    Wv = np.asarray(Wv, dtype=np.float32)
    bq = np.asarray(bq, dtype=np.float32)
    bk = np.asarray(bk, dtype=np.float32)
    bv = np.asarray(bv, dtype=np.float32)

    B, S, H = hs.shape
    assert (B, S, H) == (B_FULL, S_FULL, H_FULL), "kernel is shape-specialized"
    with_mask = bool(np.any(am))
    with_bias = bool(np.any(bq) or np.any(bk) or np.any(bv))

    nc = _build(S, H, HEADS_PER_CORE, with_mask, with_bias)

    from concourse.bass_utils import run_bass_kernel_spmd
    in_maps = _shard_inputs(hs, am, Wq, bq, Wk, bk, Wv, bv, with_mask,
                            with_bias, HEADS_PER_CORE)
    # NTFF tracing is unavailable under this axon client (antenv.axon_hooks
    # is absent); make sure an inherited BASS_TRACE can't divert the run
    # into that path.
    import os
    prev = os.environ.get("BASS_NEVER_TRACE")
    os.environ["BASS_NEVER_TRACE"] = "1"
    try:
        res = run_bass_kernel_spmd(nc, in_maps, core_ids=list(range(NCORES)))
    finally:
        if prev is None:
            os.environ.pop("BASS_NEVER_TRACE", None)
        else:
            os.environ["BASS_NEVER_TRACE"] = prev
    LAST_RESULT = res

    hd = HEADS_PER_CORE * DHEAD
    outp = np.empty((B, S, H), dtype=np.float32)
    for c in range(NCORES):
        b = c // CORES_PER_BATCH
        g = c % CORES_PER_BATCH
        outp[b, :, g * hd:(g + 1) * hd] = res.results[c]["out"]
    return outp
